# revision 2
# baseline (speedup 1.0000x reference)
"""Trainium2 Bass kernel for the batch ConsistencyLoss (masked pairwise KL).

Math (reference):
    emb = x / ||x||;  sim = emb @ emb.T;  mask = (sim > 0.8) & ~eye
    L = log_softmax(routing);  P = exp(L);  ne[j] = sum_k P[j,k] L[j,k]
    kl[i,j] = ne[j] - (L @ P.T)[i,j]
    loss = sum(mask * kl) / count(mask)

Device algorithm (per core, row strip S of 1024 rows):
  * Embeddings are transposed RAW (bf16) — no per-row normalization pass.
    Row norms come from one squared-column reduction via a ones-matmul; the
    similarity test is normalized on the threshold side instead:
        r_ij = x_i . x_j   (raw bf16 matmul)
        mask = (r_ij * (1/n_i)) > (0.8 * n_j)
    with 1/n_i per-partition ([128,1] per row chunk) and 0.8*n_j broadcast
    into a [128, B] bf16 operand — a single fused scalar_tensor_tensor per
    PSUM tile.
  * Masked-KL sum factorization:
        sum_{i in S, j} mask[i,j]*kl[i,j]
          = sum_j ne[j]*colcount_S[j] - sum_{j,k} P[j,k]*(mask_S^T @ L_S)[j,k]
    One PSUM-accumulated matmul U = [L_S|1]^T @ mask gives both terms
    (colcount in row 16).  Diagonal pairs have kl == 0 exactly, so they stay
    in the mask and the host subtracts B from the pair count.
  * Final: masked_sum = sum(W17 ⊙ U^T) with W17 = [-P | ne], one multiply +
    one reduction; per-core (sum, count) pair summed on the host.
"""

import numpy as np

import concourse.bacc as bacc
import concourse.tile as tile
from concourse import mybir
from concourse.bass_utils import run_bass_kernel_spmd
from concourse.masks import make_identity

B, E, H = 8192, 16, 1024
NCORES = 8
STRIP = B // NCORES  # 1024 rows per core
MT = STRIP // 128    # 8 row chunks per strip
KT = H // 128        # 8 contraction tiles
NT = B // 512        # 16 column tiles of 512
BT = B // 128        # 64 batch tiles
SIM_THRESHOLD = 0.8
WEIGHT = 1.0
F32 = mybir.dt.float32
BF16 = mybir.dt.bfloat16
AX = mybir.AxisListType.X
AXY = mybir.AxisListType.XY
OP = mybir.AluOpType
AF = mybir.ActivationFunctionType


def _softmax_stats(nc, pool, x, negP_out=None, ne_out=None, L_out=None):
    """From logits tile x [128, E]: optionally write -P (f32), ne (f32
    [128,1], ne = sum_k P log P) and L (any dtype) tiles."""
    negmax = pool.tile([128, 1], F32, tag="negmax")
    nc.vector.reduce_max(out=negmax, in_=x, axis=AX, negate=True)
    e = pool.tile([128, E], F32, tag="e")
    s = pool.tile([128, 1], F32, tag="s")
    nc.scalar.activation(out=e, in_=x, func=AF.Exp, bias=negmax, scale=1.0,
                         accum_out=s)
    logs = pool.tile([128, 1], F32, tag="logs")
    nc.scalar.activation(out=logs, in_=s, func=AF.Ln)
    if L_out is not None:
        # L = (x + negmax) - log(sum)
        nc.vector.tensor_scalar(L_out, x, negmax, logs, op0=OP.add,
                                op1=OP.subtract)
    if negP_out is not None:
        rs = pool.tile([128, 1], F32, tag="rs")
        nc.vector.reciprocal(out=rs, in_=s)
        nc.vector.tensor_scalar(negP_out, e, rs, -1.0, op0=OP.mult,
                                op1=OP.mult)
    if ne_out is not None:
        L = pool.tile([128, E], F32, tag="Lf")
        nc.vector.tensor_scalar(L, x, negmax, logs, op0=OP.add, op1=OP.subtract)
        scr = pool.tile([128, E], F32, tag="nescr")
        nc.vector.tensor_tensor(out=scr, in0=negP_out, in1=L, op=OP.mult)
        # scr = -P*L; negate the reduction to get ne = +sum P*L
        nc.vector.reduce_sum(out=ne_out, in_=scr, axis=AX, negate=True)


def _kernel(tc, emb, emb_s, rp, rp_s, out_dram, reps=1, loop_iters=None):
    nc = tc.nc
    with tc.tile_pool(name="persist", bufs=1) as persist:
        embt = persist.tile([128, KT, B], BF16)        # raw x^T [h%128,kt,b]
        stript = persist.tile([128, KT, STRIP], BF16)  # raw strip columns
        nb08 = persist.tile([128, B], BF16)            # 0.8*||x_j|| bcast
        rn_strip = persist.tile([128, MT], F32)        # 1/||x_i|| strip rows
        # W17[:, bt, 0:E] = -P, W17[:, bt, E] = ne — matches Ut_all layout so
        # the final masked-sum is one elementwise mult + one reduction.
        W17 = persist.tile([128, BT, E + 1], F32)
        Lpad = persist.tile([128, MT, E + 1], BF16)
        Ut_all = persist.tile([128, BT, E + 1], F32)
        identf = persist.tile([128, 128], F32)
        identb = persist.tile([128, 128], BF16)
        ones = persist.tile([128, 1], F32)
        ones_b1 = persist.tile([128, 1], BF16)
        ones_row = persist.tile([1, 128], BF16)
        make_identity(nc, identf)
        make_identity(nc, identb)
        nc.vector.memset(ones, 1.0)
        nc.vector.memset(ones_b1, 1.0)
        nc.vector.memset(ones_row, 1.0)
        nc.vector.memset(Lpad[:, :, E], 1.0)

        args = (tc, nc, emb, emb_s, rp, rp_s, out_dram, embt, stript, nb08,
                rn_strip, W17, Lpad, Ut_all, identf, identb, ones, ones_b1,
                ones_row)
        if loop_iters is not None:
            with tc.For_i(0, loop_iters, 1):
                _phases(*args, "")
            return
        for rep in range(reps):
            _phases(*args, f"r{rep}_" if reps > 1 else "")


def _phases(tc, nc, emb, emb_s, rp, rp_s, out_dram, embt, stript, nb08,
            rn_strip, W17, Lpad, Ut_all, identf, identb, ones, ones_b1,
            ones_row, r):
    # ---- Phase A: softmax stats (full batch -P/ne; strip Lpad) ----
    # All Exp ops batch under one ACT table; the 72 Ln calls collapse into
    # ONE Ln over the collected sums (ACT table loads: ~2 instead of ~99).
    TT = BT + MT
    with tc.tile_pool(name=f"{r}smx", bufs=1) as smx:
        rp_sb = smx.tile([128, BT, E], F32, tag="rp_sb")
        rps_sb = smx.tile([128, MT, E], F32, tag="rps_sb")
        nc.sync.dma_start(
            out=rp_sb, in_=rp.rearrange("(bt p) e -> p bt e", p=128))
        nc.sync.dma_start(
            out=rps_sb, in_=rp_s.rearrange("(mt p) e -> p mt e", p=128))
        e_all = smx.tile([128, TT, E], F32, tag="e_all")
        s_all = smx.tile([128, TT], F32, tag="s_all")
        nm_all = smx.tile([128, TT], F32, tag="nm_all")
        logs_all = smx.tile([128, TT], F32, tag="logs_all")
        rs_all = smx.tile([128, TT], F32, tag="rs_all")

        def logits(t):
            return rp_sb[:, t, :] if t < BT else rps_sb[:, t - BT, :]

        for t in range(TT):
            nc.vector.reduce_max(out=nm_all[:, t:t + 1], in_=logits(t),
                                 axis=AX, negate=True)
            nc.scalar.activation(out=e_all[:, t, :], in_=logits(t),
                                 func=AF.Exp, bias=nm_all[:, t:t + 1],
                                 scale=1.0, accum_out=s_all[:, t:t + 1])
        nc.scalar.activation(out=logs_all, in_=s_all, func=AF.Ln)
        nc.vector.reciprocal(out=rs_all, in_=s_all)
        for bt in range(BT):
            nc.vector.tensor_scalar(W17[:, bt, 0:E], e_all[:, bt, :],
                                    rs_all[:, bt:bt + 1], -1.0,
                                    op0=OP.mult, op1=OP.mult)
            L = smx.tile([128, E], F32, tag="Lf", bufs=2)
            nc.vector.tensor_scalar(L, rp_sb[:, bt, :], nm_all[:, bt:bt + 1],
                                    logs_all[:, bt:bt + 1], op0=OP.add,
                                    op1=OP.subtract)
            scr = smx.tile([128, E], F32, tag="nescr", bufs=2)
            nc.vector.tensor_tensor(out=scr, in0=W17[:, bt, 0:E], in1=L,
                                    op=OP.mult)
            nc.vector.reduce_sum(out=W17[:, bt, E:E + 1], in_=scr, axis=AX,
                                 negate=True)
        for ms in range(MT):
            t = BT + ms
            nc.vector.tensor_scalar(Lpad[:, ms, 0:E], rps_sb[:, ms, :],
                                    nm_all[:, t:t + 1], logs_all[:, t:t + 1],
                                    op0=OP.add, op1=OP.subtract)

    # ---- Phase B: raw transpose of embeddings (bf16) ----
    with tc.tile_pool(name=f"{r}embp", bufs=3) as ep, \
         tc.tile_pool(name=f"{r}trps", bufs=2, space="PSUM") as trps:

        def prep(src_ap, dst_tile, nb):
            for bt in range(nb):
                x = ep.tile([128, H], F32, tag="ex")
                nc.sync.dma_start(out=x,
                                  in_=src_ap[bt * 128:(bt + 1) * 128, :])
                xb = ep.tile([128, H], BF16, tag="exb", bufs=2)
                nc.scalar.copy(out=xb, in_=x)
                tp = trps.tile([128, H], BF16, tag="tr")
                for kt in range(KT):
                    nc.tensor.transpose(tp[:, kt * 128:(kt + 1) * 128],
                                        xb[:, kt * 128:(kt + 1) * 128],
                                        identb)
                nc.vector.tensor_copy(
                    out=dst_tile[:, :, bt * 128:(bt + 1) * 128],
                    in_=tp.rearrange("p (k c) -> p k c", k=KT))

        prep(emb, embt, BT)
        prep(emb_s, stript, MT)

    # ---- Phase B2: row norms via ones-matmul; nb08 + rn_strip ----
    with tc.tile_pool(name=f"{r}nrm", bufs=2) as nrm, \
         tc.tile_pool(name=f"{r}nps", bufs=2, space="PSUM") as nps, \
         tc.tile_pool(name=f"{r}bps", bufs=2, space="PSUM") as bpsp:
        # full batch: ss[j] = sum_h x[j,h]^2, nb08 = 0.8*sqrt(ss) broadcast
        for n in range(NT):
            ssp = nps.tile([1, 512], F32, tag="ssp")
            for kt in range(KT):
                sq = nrm.tile([128, 512], BF16, tag="sq")
                src = embt[:, kt, n * 512:(n + 1) * 512]
                nc.vector.tensor_tensor(out=sq, in0=src, in1=src, op=OP.mult)
                nc.tensor.matmul(out=ssp, lhsT=ones_b1, rhs=sq,
                                 start=(kt == 0), stop=(kt == KT - 1))
            n08c = nrm.tile([1, 512], BF16, tag="n08c")
            # 0.8*sqrt(ss) = sqrt(0.64*ss)
            nc.scalar.activation(out=n08c, in_=ssp, func=AF.Sqrt, bias=0.0,
                                 scale=SIM_THRESHOLD * SIM_THRESHOLD)
            bps = bpsp.tile([128, 512], F32, tag="bps")
            nc.tensor.matmul(out=bps, lhsT=ones_row, rhs=n08c, start=True,
                             stop=True)
            nc.scalar.copy(out=nb08[:, n * 512:(n + 1) * 512], in_=bps)
        # strip rows: rn_strip[p, m] = 1/||x_{m*128+p}||
        for sn in range(STRIP // 512):
            ssp = nps.tile([1, 512], F32, tag="ssp")
            for kt in range(KT):
                sq = nrm.tile([128, 512], BF16, tag="sq")
                src = stript[:, kt, sn * 512:(sn + 1) * 512]
                nc.vector.tensor_tensor(out=sq, in0=src, in1=src, op=OP.mult)
                nc.tensor.matmul(out=ssp, lhsT=ones_b1, rhs=sq,
                                 start=(kt == 0), stop=(kt == KT - 1))
            nrow = nrm.tile([1, 512], F32, tag="nrow")
            nc.scalar.activation(out=nrow, in_=ssp, func=AF.Sqrt, bias=0.0,
                                 scale=1.0)
            rrow = nrm.tile([1, 512], F32, tag="rrow")
            nc.vector.reciprocal(out=rrow, in_=nrow)
            rps = bpsp.tile([128, 4], F32, tag="rps")
            for mm in range(4):
                nc.tensor.transpose(rps[:, mm:mm + 1],
                                    rrow[:, mm * 128:(mm + 1) * 128],
                                    identf[:1, :1])
            nc.vector.tensor_copy(out=rn_strip[:, sn * 4:(sn + 1) * 4],
                                  in_=rps)

    # ---- Phase C: raw sim matmul + fused normalize-threshold + U ----
    with tc.tile_pool(name=f"{r}simps", bufs=3, space="PSUM") as sps, \
         tc.tile_pool(name=f"{r}ups", bufs=2, space="PSUM") as ups, \
         tc.tile_pool(name=f"{r}utps", bufs=2, space="PSUM") as utps, \
         tc.tile_pool(name=f"{r}mkp", bufs=3) as mkp, \
         tc.tile_pool(name=f"{r}stg", bufs=2) as stg:
        for n in range(NT):
            u = ups.tile([E + 1, 512], F32, tag="u")
            for m in range(MT):
                sim = sps.tile([128, 512], F32, tag="sim")
                for kt in range(KT):
                    nc.tensor.matmul(
                        out=sim,
                        lhsT=stript[:, kt, m * 128:(m + 1) * 128],
                        rhs=embt[:, kt, n * 512:(n + 1) * 512],
                        start=(kt == 0), stop=(kt == KT - 1))
                msk = mkp.tile([128, 512], BF16, tag="mask")
                # mask = (r * 1/n_i) > 0.8*n_j
                nc.vector.scalar_tensor_tensor(
                    out=msk, in0=sim, scalar=rn_strip[:, m:m + 1],
                    in1=nb08[:, n * 512:(n + 1) * 512],
                    op0=OP.mult, op1=OP.is_gt)
                nc.tensor.matmul(out=u, lhsT=Lpad[:, m, :], rhs=msk,
                                 start=(m == 0), stop=(m == MT - 1))
            # stage U and transpose 128-column blocks into Ut_all
            ust = stg.tile([E + 1, 512], F32, tag="ust")
            nc.scalar.copy(out=ust, in_=u)
            for c in range(4):
                jt = n * 4 + c
                tp = utps.tile([128, E + 1], F32, tag="ut")
                nc.tensor.matmul(out=tp,
                                 lhsT=ust[:, c * 128:(c + 1) * 128],
                                 rhs=identf[:E + 1, :E + 1],
                                 start=True, stop=True)
                if c % 2 == 0:
                    nc.vector.tensor_copy(out=Ut_all[:, jt, :], in_=tp)
                else:
                    nc.scalar.copy(out=Ut_all[:, jt, :], in_=tp)

    # ---- Phase D: final reduction to (masked_sum, count) ----
    with tc.tile_pool(name=f"{r}fin", bufs=1) as fin, \
         tc.tile_pool(name=f"{r}fps", bufs=1, space="PSUM") as fps:
        scr = fin.tile([128, BT, E + 1], F32)
        nc.vector.tensor_tensor(out=scr, in0=W17, in1=Ut_all, op=OP.mult)
        accs = fin.tile([128, 2], F32)
        nc.vector.reduce_sum(out=accs[:, 0:1], in_=scr, axis=AXY)
        nc.vector.reduce_sum(out=accs[:, 1:2], in_=Ut_all[:, :, E:E + 1],
                             axis=AXY)
        res = fps.tile([1, 2], F32)
        nc.tensor.matmul(out=res, lhsT=ones, rhs=accs, start=True, stop=True)
        out_sb = fin.tile([1, 2], F32)
        nc.scalar.copy(out=out_sb, in_=res)
        nc.sync.dma_start(out=out_dram, in_=out_sb)


def build_bass(reps=1, loop_iters=None):
    nc = bacc.Bacc("TRN2", target_bir_lowering=False, debug=False)
    emb = nc.dram_tensor("emb", [B, H], F32, kind="ExternalInput").ap()
    emb_s = nc.dram_tensor("emb_strip", [STRIP, H], F32,
                           kind="ExternalInput").ap()
    rp = nc.dram_tensor("rp", [B, E], F32, kind="ExternalInput").ap()
    rp_s = nc.dram_tensor("rp_strip", [STRIP, E], F32,
                          kind="ExternalInput").ap()
    out = nc.dram_tensor("out", [1, 2], F32, kind="ExternalOutput").ap()
    with tile.TileContext(nc) as tc:
        _kernel(tc, emb, emb_s, rp, rp_s, out, reps=reps,
                loop_iters=loop_iters)
    nc.compile()
    return nc


_NC_CACHE = None


def make_in_map(rp: np.ndarray, emb: np.ndarray, d: int) -> dict:
    return {
        "emb": emb,
        "emb_strip": np.ascontiguousarray(emb[d * STRIP:(d + 1) * STRIP]),
        "rp": rp,
        "rp_strip": np.ascontiguousarray(rp[d * STRIP:(d + 1) * STRIP]),
    }


def kernel(routing_probs: np.ndarray, input_embeddings: np.ndarray,
           **_unused) -> np.ndarray:
    global _NC_CACHE
    if _NC_CACHE is None:
        _NC_CACHE = build_bass()
    nc = _NC_CACHE
    rp = np.ascontiguousarray(routing_probs, dtype=np.float32)
    emb = np.ascontiguousarray(input_embeddings, dtype=np.float32)
    in_maps = []
    for d in range(NCORES):
        in_maps.append({
            "emb": emb,
            "emb_strip": np.ascontiguousarray(emb[d * STRIP:(d + 1) * STRIP]),
            "rp": rp,
            "rp_strip": np.ascontiguousarray(rp[d * STRIP:(d + 1) * STRIP]),
        })
    res = run_bass_kernel_spmd(nc, in_maps, core_ids=list(range(NCORES)))
    vals = np.array([r["out"].reshape(2) for r in res.results],
                    dtype=np.float64)
    total = vals[:, 0].sum()
    cnt = vals[:, 1].sum() - B  # drop the diagonal pairs (kl there is 0)
    if cnt > 0:
        loss = np.float32(total) / np.float32(max(cnt, 1.0))
    else:
        loss = 0.0
    return np.array(WEIGHT * loss, dtype=np.float32)



# revision 19
# speedup vs baseline: 2.7027x; 2.7027x over previous
"""Trainium2 Bass kernel for the batch ConsistencyLoss (masked pairwise KL).

Math (reference):
    emb = x / ||x||;  sim = emb @ emb.T;  mask = (sim > 0.8) & ~eye
    L = log_softmax(routing);  P = exp(L);  ne[j] = sum_k P[j,k] L[j,k]
    kl[i,j] = ne[j] - L_i . P_j
    loss = sum(mask * kl) / count(mask)

Key ideas vs a row-strip baseline:
  * Upper-triangle only: mask is symmetric and
        kl[i,j] + kl[j,i] = ne_i + ne_j - L_i.P_j - P_i.L_j
    is symmetric in (i,j), so each unordered block pair {a,b} of the
    16x512-row block grid is computed once, halving the dominant
    sim matmul.  Per-core assignment (8 cores x 17 block tasks) is made
    SPMD-uniform by gathering per-core block lists on the host into one
    input; every core runs the identical program over local positions.
  * Both matmul operands are scaled by C/||x|| during the f32->fp8
    conversion, so sim PSUM holds C^2*cos directly and the mask is a
    single constant-threshold compare (no norm broadcasts).
  * fp8(e4m3) DoubleRow matmuls: 2 k-subtiles per instruction.
  * Masked-KL factorization per task via one [34-wide] matmul:
        U = [L|P|ne|1]_rows^T @ mask  ->  S += sum_j F_cols (x) U^T
    with F = [-P|-L|1|ne]; pair count rides along as U's last row.
"""

import numpy as np

import concourse.bacc as bacc
import concourse.tile as tile
from concourse import mybir
from concourse.bass_utils import run_bass_kernel_spmd
from concourse.masks import make_identity

B, E, H = 8192, 16, 1024
NCORES = 8
NB = 16          # 512-row blocks of the batch
BS = 512
KT = H // 128    # 8 contraction chunks of 128
GN = 11          # gathered 512-row groups per core
GC = GN * 4      # 44 chunks of 128 rows
GROWS = GN * BS  # 5632 gathered rows
CS = 16.0        # fp8 scale: rows scaled by CS/||x||
THR = 0.8 * CS * CS
WEIGHT = 1.0

F32 = mybir.dt.float32
BF16 = mybir.dt.bfloat16
FP8 = mybir.dt.float8e4
I32 = mybir.dt.int32
AX = mybir.AxisListType.X
AXY = mybir.AxisListType.XY
OP = mybir.AluOpType
AF = mybir.ActivationFunctionType
DR = mybir.MatmulPerfMode.DoubleRow

# 17 uniform tasks over local group positions (rows_pos, cols_pos, is_diag),
# ordered so early-prepped groups unblock compute first.
TASKS = [(0, 0, True), (0, 1, False), (1, 1, True)]
for _c in range(2, 8):
    TASKS += [(0, _c, False), (1, _c, False)]
TASKS += [(1, 8, False), (9, 10, False)]
assert len(TASKS) == 17
GB_BASE = {0: 0, 1: 4, 9: 8}  # rows position -> Gb chunk base


def core_blocks(d: int) -> list:
    blocks = [(2 * d + o) % NB for o in range(9)]
    if d < 4:
        a, b = 2 * d, (2 * d + 8) % NB
    else:
        a, b = (2 * d + 1) % NB, (2 * d + 9) % NB
    return blocks + [a, b]


def _check_coverage():
    seen = {}
    for d in range(NCORES):
        bl = core_blocks(d)
        for (pr, pc, diag) in TASKS:
            assert diag == (bl[pr] == bl[pc])
            key = (min(bl[pr], bl[pc]), max(bl[pr], bl[pc]))
            seen[key] = seen.get(key, 0) + 1
    assert sorted(seen) == [(a, b) for a in range(NB) for b in range(a, NB)]
    assert all(v == 1 for v in seen.values())


_check_coverage()


def _kernel(tc, embg, rpg, out_dram, reps=1, loop_iters=None, upto="Z"):
    nc = tc.nc
    with tc.tile_pool(name="persist", bufs=1) as persist:
        embt = persist.tile([128, KT, GROWS], FP8)   # scaled emb^T chunks
        F_all = persist.tile([128, GC, 34], F32)     # [-P|-L|1|ne] per row
        Gb = persist.tile([128, 12, 34], BF16)       # [L|P|ne|1] row groups
        Ut_all = persist.tile([128, 68, 34], F32)    # U^T per task j-chunk
        Q = persist.tile([128, 896], BF16)           # strict-upper patterns
        io = persist.tile([128, 896], I32)
        identb = persist.tile([128, 128], BF16)
        identf = persist.tile([128, 128], F32)
        ones = persist.tile([128, 1], F32)
        ss = persist.tile([128, GC], F32)            # sum of squares / row
        nrm = persist.tile([128, GC], F32)
        rn = persist.tile([128, GC], F32)            # CS / ||x||

        make_identity(nc, identb)
        make_identity(nc, identf)
        nc.vector.memset(ones, 1.0)
        nc.gpsimd.iota(io, pattern=[[1, 896]], base=-384,
                       channel_multiplier=-1)
        nc.vector.tensor_scalar(Q, io, 0, None, op0=OP.is_gt)
        nc.vector.memset(F_all[:, :, 32:33], 1.0)
        nc.vector.memset(Gb[:, :, 33:34], 1.0)

        args = (tc, nc, embg, rpg, out_dram, embt, F_all, Gb, Ut_all, Q,
                identb, identf, ones, ss, nrm, rn)
        if loop_iters is not None:
            with tc.For_i(0, loop_iters, 1):
                _phases(*args, "", upto)
            return
        for rep in range(reps):
            _phases(*args, f"r{rep}_" if reps > 1 else "", upto)


def _phases(tc, nc, embg, rpg, out_dram, embt, F_all, Gb, Ut_all, Q,
            identb, identf, ones, ss, nrm, rn, r, upto="Z"):
    # ---- Phase A: softmax stats for all 44 gathered chunks ----
    with tc.tile_pool(name=f"{r}smx", bufs=1) as smx:
        rp_sb = smx.tile([128, GC, E], F32, tag="rp_sb")
        nc.sync.dma_start(
            out=rp_sb, in_=rpg.rearrange("(c p) e -> p c e", p=128))
        e_all = smx.tile([128, GC, E], F32, tag="e_all")
        s_all = smx.tile([128, GC], F32, tag="s_all")
        logs_all = smx.tile([128, GC], F32, tag="logs_all")
        rs_all = smx.tile([128, GC], F32, tag="rs_all")
        for c in range(GC):
            nc.scalar.activation(out=e_all[:, c, :], in_=rp_sb[:, c, :],
                                 func=AF.Exp, bias=0.0, scale=1.0,
                                 accum_out=s_all[:, c:c + 1])
        nc.scalar.activation(out=logs_all, in_=s_all, func=AF.Ln)
        nc.vector.reciprocal(out=rs_all, in_=s_all)
        for c in range(GC):
            # F[:, c, 0:16] = -P;  F[:, c, 16:32] = -L
            nc.vector.tensor_scalar(F_all[:, c, 0:16], e_all[:, c, :],
                                    rs_all[:, c:c + 1], -1.0,
                                    op0=OP.mult, op1=OP.mult)
            nc.vector.tensor_scalar(F_all[:, c, 16:32], rp_sb[:, c, :],
                                    logs_all[:, c:c + 1], -1.0,
                                    op0=OP.subtract, op1=OP.mult)
        # ne = sum_k P*L = sum (-P)(-L), batched over all chunks
        scr3 = smx.tile([128, GC, E], F32, tag="scr3")
        nc.vector.tensor_tensor(out=scr3, in0=F_all[:, :, 0:16],
                                in1=F_all[:, :, 16:32], op=OP.mult)
        ne_t = smx.tile([128, GC, 1], F32, tag="ne_t")
        nc.vector.reduce_sum(out=ne_t, in_=scr3, axis=AX)
        nc.vector.tensor_copy(out=F_all[:, :, 33:34], in_=ne_t)
        # Gb rows: positions {0,1,9} -> chunks {0..7, 36..39}
        for rr in range(12):
            c = rr if rr < 8 else 28 + rr
            nc.vector.tensor_scalar(Gb[:, rr, 0:16], rp_sb[:, c, :],
                                    logs_all[:, c:c + 1], None,
                                    op0=OP.subtract)
            nc.vector.tensor_scalar(Gb[:, rr, 16:32], e_all[:, c, :],
                                    rs_all[:, c:c + 1], None, op0=OP.mult)
        nc.vector.tensor_copy(out=Gb[:, 0:8, 32:33], in_=F_all[:, 0:8, 33:34])
        nc.vector.tensor_copy(out=Gb[:, 8:12, 32:33],
                              in_=F_all[:, 36:40, 33:34])

    if upto == "A":
        with tc.tile_pool(name=f"{r}dbg", bufs=1) as dbg:
            a2 = dbg.tile([128, 2], F32)
            nc.vector.reduce_sum(out=a2[:, 0:1], in_=F_all, axis=AXY)
            nc.vector.reduce_sum(out=a2[:, 1:2], in_=Gb, axis=AXY)
            nc.sync.dma_start(out=out_dram, in_=a2[0:1, :])
        return

    # ---- Phases B+C+D under shared PSUM pools so they can overlap ----
    with tc.tile_pool(name=f"{r}prep", bufs=1) as prep, \
         tc.tile_pool(name=f"{r}task", bufs=1) as taskp, \
         tc.tile_pool(name=f"{r}trps", bufs=2, space="PSUM") as trps, \
         tc.tile_pool(name=f"{r}sps", bufs=2, space="PSUM") as sps, \
         tc.tile_pool(name=f"{r}ups", bufs=2, space="PSUM") as ups, \
         tc.tile_pool(name=f"{r}utps", bufs=1, space="PSUM") as utps:

        # ---- B: DMA -> sum-squares -> scale+convert -> transpose ----
        for g in range(GN):
            xs = []
            for cc in range(4):
                c = 4 * g + cc
                x = prep.tile([128, H], F32, tag="x", bufs=6)
                xs.append(x)
                nc.sync.dma_start(out=x, in_=embg[c * 128:(c + 1) * 128, :])
                sqs = prep.tile([128, H], F32, tag="sqs", bufs=2)
                nc.scalar.activation(out=sqs, in_=x, func=AF.Square,
                                     bias=0.0, scale=1.0,
                                     accum_out=ss[:, c:c + 1])
            # rn = CS/||x|| per group of 4 chunks
            g4 = slice(4 * g, 4 * g + 4)
            nc.scalar.activation(out=nrm[:, g4], in_=ss[:, g4], func=AF.Sqrt,
                                 bias=0.0, scale=1.0 / (CS * CS))
            nc.vector.reciprocal(out=rn[:, g4], in_=nrm[:, g4])
            for cc in range(4):
                c = 4 * g + cc
                xb = prep.tile([128, H], BF16, tag="xb", bufs=2)
                nc.vector.tensor_scalar(xb, xs[cc], rn[:, c:c + 1], None,
                                        op0=OP.mult)
                tp = trps.tile([128, H], BF16, tag="tp")
                for kt in range(KT):
                    nc.tensor.transpose(tp[:, kt * 128:(kt + 1) * 128],
                                        xb[:, kt * 128:(kt + 1) * 128],
                                        identb)
                nc.vector.tensor_copy(
                    out=embt[:, :, c * 128:(c + 1) * 128],
                    in_=tp.rearrange("p (k q) -> p k q", k=KT))

        if upto == "B":
            with tc.tile_pool(name=f"{r}dbg", bufs=1) as dbg:
                a2 = dbg.tile([128, 2], F32)
                e8 = dbg.tile([128, KT, 128], F32)
                nc.vector.tensor_copy(out=e8, in_=embt[:, :, 0:128])
                nc.vector.reduce_sum(out=a2[:, 0:1], in_=e8, axis=AXY)
                nc.vector.reduce_sum(out=a2[:, 1:2], in_=ss, axis=AX)
                nc.sync.dma_start(out=out_dram, in_=a2[0:1, :])
            return

        # ---- C: 17 tasks: fp8 DR sim -> mask -> U matmul -> U^T ----
        for t, (pr, pc, diag) in enumerate(TASKS):
            u = ups.tile([34, 512], F32, tag="u")
            for m in range(4):
                rs0 = (pr * 4 + m) * 128
                cs0 = pc * 512
                sim = sps.tile([128, 512], F32, tag="sim")
                for t2 in range(4):
                    nc.tensor.matmul(
                        out=sim,
                        lhsT=embt[:, 2 * t2:2 * t2 + 2, rs0:rs0 + 128],
                        rhs=embt[:, 2 * t2:2 * t2 + 2, cs0:cs0 + 512],
                        start=(t2 == 0), stop=(t2 == 3), perf_mode=DR)
                msk = taskp.tile([128, 512], BF16, tag="msk", bufs=3)
                if diag:
                    off = (3 - m) * 128
                    nc.vector.scalar_tensor_tensor(
                        out=msk, in0=sim, scalar=THR,
                        in1=Q[:, off:off + 512], op0=OP.is_gt, op1=OP.mult)
                else:
                    nc.vector.tensor_scalar(msk, sim, THR, None,
                                            op0=OP.is_gt)
                nc.tensor.matmul(out=u, lhsT=Gb[:, GB_BASE[pr] + m, :],
                                 rhs=msk, start=(m == 0), stop=(m == 3))
            ust = taskp.tile([34, 512], F32, tag="ust", bufs=2)
            nc.scalar.copy(out=ust, in_=u)
            for q in range(4):
                utp = utps.tile([128, 34], F32, tag="utp")
                nc.tensor.matmul(out=utp, lhsT=ust[:, q * 128:(q + 1) * 128],
                                 rhs=identf[:34, :34], start=True, stop=True)
                nc.scalar.copy(out=Ut_all[:, 4 * t + q, :], in_=utp)
            if upto.startswith("C") and t == int(upto[1:] or 0):
                break

        if upto.startswith("C"):
            with tc.tile_pool(name=f"{r}dbg", bufs=1) as dbg:
                a2 = dbg.tile([128, 2], F32)
                nc.vector.reduce_sum(out=a2[:, 0:1], in_=Ut_all[:, 0:4, :],
                                     axis=AXY)
                nc.vector.reduce_sum(out=a2[:, 1:2], in_=ss, axis=AX)
                nc.sync.dma_start(out=out_dram, in_=a2[0:1, :])
            return

        # ---- D: final reduction to (S_part, C_u) ----
        with tc.tile_pool(name=f"{r}fin", bufs=1) as fin:
            accs = fin.tile([128, 17], F32)
            for t, (pr, pc, diag) in enumerate(TASKS):
                # NB: tensor_tensor_reduce wedges the device in this flow;
                # use TT + reduce instead.
                scrT = fin.tile([128, 4, 34], F32, tag="scrT", bufs=2)
                nc.vector.tensor_tensor(
                    out=scrT, in0=F_all[:, pc * 4:pc * 4 + 4, :],
                    in1=Ut_all[:, 4 * t:4 * t + 4, :], op=OP.mult)
                nc.vector.reduce_sum(out=accs[:, t:t + 1], in_=scrT,
                                     axis=AXY)
            accs2 = fin.tile([128, 2], F32)
            nc.vector.reduce_sum(out=accs2[:, 0:1], in_=accs, axis=AX)
            nc.vector.reduce_sum(out=accs2[:, 1:2], in_=Ut_all[:, :, 33:34],
                                 axis=AXY)
            res = utps.tile([1, 2], F32, tag="res")
            nc.tensor.matmul(out=res, lhsT=ones, rhs=accs2, start=True,
                             stop=True)
            out_sb = fin.tile([1, 2], F32)
            nc.scalar.copy(out=out_sb, in_=res)
            nc.sync.dma_start(out=out_dram, in_=out_sb)


def build_bass(reps=1, loop_iters=None, upto="Z"):
    nc = bacc.Bacc("TRN2", target_bir_lowering=False, debug=False)
    embg = nc.dram_tensor("embg", [GROWS, H], F32, kind="ExternalInput").ap()
    rpg = nc.dram_tensor("rpg", [GROWS, E], F32, kind="ExternalInput").ap()
    out = nc.dram_tensor("out", [1, 2], F32, kind="ExternalOutput").ap()
    with tile.TileContext(nc) as tc:
        _kernel(tc, embg, rpg, out, reps=reps, loop_iters=loop_iters,
                upto=upto)
    nc.compile()
    return nc


_NC_CACHE = None


def make_in_map(rp: np.ndarray, emb: np.ndarray, d: int) -> dict:
    ev = emb.reshape(NB, BS, H)
    rv = rp.reshape(NB, BS, E)
    bl = core_blocks(d)
    return {
        "embg": np.concatenate([ev[g] for g in bl], 0),
        "rpg": np.concatenate([rv[g] for g in bl], 0),
    }


def kernel(routing_probs: np.ndarray, input_embeddings: np.ndarray,
           **_unused) -> np.ndarray:
    global _NC_CACHE
    if _NC_CACHE is None:
        _NC_CACHE = build_bass()
    nc = _NC_CACHE
    rp = np.ascontiguousarray(routing_probs, dtype=np.float32)
    emb = np.ascontiguousarray(input_embeddings, dtype=np.float32)
    in_maps = [make_in_map(rp, emb, d) for d in range(NCORES)]
    res = run_bass_kernel_spmd(nc, in_maps, core_ids=list(range(NCORES)))
    vals = np.array([r["out"].reshape(2) for r in res.results],
                    dtype=np.float64)
    s_tot = vals[:, 0].sum()
    cnt = vals[:, 1].sum()  # unordered masked pairs
    if cnt > 0:
        loss = np.float32(s_tot) / np.float32(2.0 * cnt)
    else:
        loss = 0.0
    return np.array(WEIGHT * loss, dtype=np.float32)
